# revision 2
# baseline (speedup 1.0000x reference)
"""Causal self-attention (B=1, S=4096, D=1024, 16 heads) on 8 trn2 NeuronCores.

Sharding: tensor-parallel over heads (2 heads per core). Each core computes
the qkv projection for its head pair, causal attention, and a partial output
projection; the host sums the 8 fp16 partials (plus an f32 side-output for
rows 0-127) and adds b_out.

Design (vs the fp32r baseline):
- The attention j-loop is ACT-engine-gated (exp ~1.04us/tile vs PE ~0.65us),
  so PV runs TRANSPOSED in fp16: ctx^T accumulators [128q, 65] per q-block,
  moving operand = [V|1] (65 cols). This halves PV tensor rows AND puts the
  softmax denominator on the partition axis, so normalization is a plain
  per-partition tensor_scalar_mul (no partition broadcast / gpsimd chain).
- PE is in-order; projection matmuls of chunk c+1 and the output projection
  of chunk c (per q-block, as each lands) are queued as background closures
  drained between j-iterations to fill PE's exp-wait gaps. Group-selective
  barriers force-drain only what correctness needs (q before the j-loop,
  k/v before the diagonal, outproj before its buffer rings recycle).
- q/k/v/attention-weights are fp16 (fp32 PSUM accumulation everywhere;
  fp16's 10-bit mantissa matmuls run at full PE rate). Rows 0-127 attend
  few keys so |ctx| has no averaging decay there: q-block (chunk0, s=0)
  runs an f32/f32r path end-to-end and is emitted via a separate f32
  output that the host prefers.
- PSUM: start_tensor_calc zeroes a whole 2KB bank, so the 8 packed ctx^T
  accumulators get exactly one start per bank; diagonal masking runs on
  gpsimd (affine_select) off the critical engines.
"""
import sys

sys.path.insert(0, "/opt/trn_rl_repo")

from contextlib import ExitStack

import numpy as np

import concourse.tile as tile
from concourse import bacc, mybir
from concourse.alu_op_type import AluOpType
from concourse.masks import make_identity
from concourse.bass_utils import run_bass_kernel_spmd

D = 1024
N_CORES = 8
F32 = mybir.dt.float32
F32R = mybir.dt.float32r
F16 = mybir.dt.float16
AF = mybir.ActivationFunctionType
F16NP = np.float16

QC = 512  # q-chunk width
KT = 128  # k-tile width


def build_program(S: int = 4096, repeat: int = 1):
    nqc = S // QC

    nc = bacc.Bacc(None)
    xT = nc.declare_dram_parameter("xT", [D, S], F32R, isOutput=False)
    w_sh = nc.declare_dram_parameter("w_sh", [D, 384], F32R, isOutput=False)
    b_sh = nc.declare_dram_parameter("b_sh", [384], F32, isOutput=False)
    w_o = nc.declare_dram_parameter("w_o", [128, D], F16, isOutput=False)
    w_o32 = nc.declare_dram_parameter("w_o32", [128, D], F32R, isOutput=False)
    outp = nc.declare_dram_parameter("outp", [S, D], F16, isOutput=True)
    outp32 = nc.declare_dram_parameter("outp32", [128, D], F32, isOutput=True)

    with tile.TileContext(nc) as tc, ExitStack() as ctx:
        consts = ctx.enter_context(tc.tile_pool(name="consts", bufs=1))
        big = ctx.enter_context(tc.tile_pool(name="big", bufs=1))
        xpool = ctx.enter_context(tc.tile_pool(name="xp", bufs=4))
        vtpool = ctx.enter_context(tc.tile_pool(name="vt", bufs=2))
        stpool = ctx.enter_context(tc.tile_pool(name="st", bufs=3))
        apool = ctx.enter_context(tc.tile_pool(name="at", bufs=6))
        npool = ctx.enter_context(tc.tile_pool(name="nrm", bufs=2))
        opool = ctx.enter_context(tc.tile_pool(name="ot", bufs=3))
        psS = ctx.enter_context(tc.tile_pool(name="psS", bufs=2, space="PSUM"))
        psCtx = ctx.enter_context(tc.tile_pool(name="psCtx", bufs=1, space="PSUM"))
        psO = ctx.enter_context(tc.tile_pool(name="psO", bufs=2, space="PSUM"))

        # ---- constants
        ident_f = consts.tile([128, 128], F32)
        make_identity(nc, ident_f[:])
        ident = consts.tile([128, 128], F16)
        nc.vector.tensor_copy(ident[:], ident_f[:])

        ones_bf = consts.tile([128, 8], F16)
        nc.gpsimd.memset(ones_bf[:], 1.0)

        # load order = first-use order: w_sb m0, biases, then m1/m2/w_o
        w_sb = consts.tile([128, 8, 384], F32R)
        biases = consts.tile([128, 3], F32)
        nc.sync.dma_start(
            w_sb[:, :, 0:128],
            w_sh.rearrange("(t p) m -> p t m", p=128)[:, :, 0:128],
        )
        nc.sync.dma_start(biases[:], b_sh.rearrange("(m p) -> p m", p=128))
        for m in (1, 2):
            nc.sync.dma_start(
                w_sb[:, :, m * 128:(m + 1) * 128],
                w_sh.rearrange("(t p) m -> p t m", p=128)[:, :, m * 128:(m + 1) * 128],
            )
        w_o_sb = consts.tile([128, D], F16)
        nc.sync.dma_start(w_o_sb[:], w_o[:])
        w_o32_sb = consts.tile([128, D], F32R)
        nc.sync.dma_start(w_o32_sb[:], w_o32[:])

        # per-chunk projection tiles (separate tags so attention on chunk c
        # only depends on projections of chunks <= c)
        # qk_t[n]: [64p(hd), head, q|k, seq]   v_t[n]: [128p(k), s, (head, 65)]
        qk_t = [
            big.tile([64, 2, 2, QC], F16, tag=f"qk{n}", name=f"qk{n}")
            for n in range(nqc)
        ]
        v_t = [
            big.tile([128, 4, 130], F16, tag=f"v{n}", name=f"v{n}")
            for n in range(nqc)
        ]
        # rows 0-127 attend few keys, so their |ctx| has no averaging decay
        # and fp16 quantization of the output path is the dominant error:
        # q-block (chunk 0, s=0) runs a full f32/f32r path instead
        v32_t = big.tile([128, 130], F32, name="v32t")
        nc.gpsimd.memset(
            v32_t[:].rearrange("p (g c) -> p g c", g=2)[:, :, 64:65], 1.0
        )
        for n in range(nqc):
            nc.vector.tensor_copy(
                v_t[n][:].rearrange("p t (g c) -> p t g c", g=2)[:, :, :, 64:65],
                ones_bf[:].rearrange("p (t g o) -> p t g o", g=2, o=1),
            )

        for _rep in range(repeat):
            # global background-work queue: (key, closure). The j-loop drains
            # a couple of items per iteration (filling PE's exp-wait gaps);
            # barriers force-drain through a key when correctness requires it.
            bgq = []
            done_keys = set()

            def drain_n(k):
                for _ in range(k):
                    if not bgq:
                        return
                    group, sub, f = bgq.pop(0)
                    f()
                    if sub is not None:
                        done_keys.add((group, sub))

            def drain_upto(group, sub):
                """Run queued items of ONE group (in their relative order)
                through `sub`, leaving other groups queued. All items are
                data-independent across groups, and PSUM ring slots are
                allocated lazily at call time, so out-of-FIFO draining is
                safe."""
                if (group, sub) in done_keys:
                    return
                i = 0
                while i < len(bgq):
                    g, s2, f = bgq[i]
                    if g != group:
                        i += 1
                        continue
                    bgq.pop(i)
                    f()
                    if s2 is not None:
                        done_keys.add((g, s2))
                    if s2 == sub:
                        return

            def emit_xt_loads(n):
                xts = []
                src = xT.rearrange("(t p) s -> p t s", p=128)
                # chunk 0 gates the program head: split its loads finer so
                # the first projection matmul starts sooner (and keep SP
                # free for the weight loads)
                nsplit = 4 if n == 0 else 2
                for half in range(2):
                    xt = xpool.tile([128, 4, QC], F32R)
                    w = 4 // nsplit
                    for q in range(nsplit):
                        eng = nc.sync if (q % 2 == 0 and n > 0) else nc.gpsimd
                        eng.dma_start(
                            xt[:, w * q:w * (q + 1), :],
                            src[:, 4 * half + w * q:4 * half + w * (q + 1),
                                n * QC:(n + 1) * QC],
                        )
                    xts.append(xt)
                return xts

            def push_proj(n, xts):
                """Queue the projection of chunk n as individually drainable
                closures (one PE matmul each; DVE/transpose work attached to
                the last closure of each m-block). PSUM tiles are allocated
                lazily at call time so ring-slot order == emission order."""
                stage = stpool.tile([128, 2, QC], F16)
                cell = {}
                for m in range(3):
                    for t in range(8):
                        def mm(m=m, t=t):
                            if t == 0:
                                cell[m] = psO.tile(
                                    [128, QC], F32, tag="mm512", name="psproj"
                                )
                            nc.tensor.matmul(
                                cell[m][:],
                                w_sb[:, t, m * 128:(m + 1) * 128],
                                xts[t // 4][:, t % 4, :],
                                start=(t == 0),
                                stop=(t == 7),
                                skip_group_check=True,
                            )
                        bgq.append((("p", n), None, mm))

                    def tail(m=m):
                        ps = cell[m]
                        if m < 2:
                            nc.vector.tensor_scalar_add(
                                qk_t[n][:, 0, m, :], ps[0:64, :],
                                biases[0:64, m:m + 1],
                            )
                            nc.vector.tensor_scalar_add(
                                stage[64:128, m, :], ps[64:128, :],
                                biases[64:128, m:m + 1],
                            )
                            # relocate head B to partitions 0-63 right away:
                            # q (m0) unblocks the j-loop without waiting for k
                            nc.gpsimd.dma_start(
                                qk_t[n][:, 1, m:m + 1, :],
                                stage[64:128, m:m + 1, :],
                            )
                        else:
                            vt_c = vtpool.tile([128, QC], F16)
                            nc.vector.tensor_scalar_add(
                                vt_c[:], ps[:], biases[:, 2:3]
                            )
                            tr = psO.tile(
                                [128, 4, 128], F16, tag="mm512", name="tr"
                            )
                            for s in range(4):
                                nc.tensor.transpose(
                                    tr[:, s, :],
                                    vt_c[:, s * 128:(s + 1) * 128],
                                    ident[:],
                                )
                            nc.vector.tensor_copy(
                                v_t[n][:].rearrange(
                                    "p t (g c) -> p t g c", g=2
                                )[:, :, :, 0:64],
                                tr[:].rearrange("p t (g c) -> p t g c", g=2),
                            )
                            if n == 0:
                                # f32 V of k-tile 0 for the f32 (c0,s0) path
                                vt32_c = vtpool.tile(
                                    [128, 128], F32, tag="vt32", name="vt32"
                                )
                                nc.vector.tensor_scalar_add(
                                    vt32_c[:], ps[:, 0:128], biases[:, 2:3]
                                )
                                tr32 = psO.tile(
                                    [128, 128], F32, tag="mm512", name="tr32"
                                )
                                nc.tensor.transpose(
                                    tr32[:], vt32_c[:], ident_f[:]
                                )
                                nc.vector.tensor_copy(
                                    v32_t[:].rearrange(
                                        "p (g c) -> p g c", g=2
                                    )[:, :, 0:64],
                                    tr32[:].rearrange("p (g c) -> p g c", g=2),
                                )
                    bgq.append((("p", n), m, tail))

            def push_outproj_s(c, ctxnT, s, ost, ctxnT32=None):
                """Queue the transpose-back + output projection of q-block s
                of chunk c (emitted as block s's normalization lands, so the
                last chunk's output drains during its own j-loop). Block
                (c=0, s=0) uses the f32/f32r path."""
                f32path = ctxnT32 is not None

                def tback(s=s):
                    if "ot" not in ost:
                        ost["ot"] = opool.tile([128, 4, D], F16, name="ot")
                    if f32path:
                        ost["ot32"] = opool.tile(
                            [128, D], F32, tag="ot32", name="ot32"
                        )
                        t32 = psO.tile(
                            [128, 128], F32, tag="mm512", name="ctxnps32"
                        )
                        nc.tensor.transpose(t32[:], ctxnT32[:], ident_f[:])
                        ost["sb32"] = npool.tile(
                            [128, 128], F32R, tag="ctxn32", name="ctxnsb32"
                        )
                        nc.vector.tensor_copy(ost["sb32"][:], t32[:])
                        return
                    if "t" not in ost:
                        ost["t"] = psO.tile(
                            [128, 4, 128], F16, tag="mm512", name="ctxnps"
                        )
                        ost["sb"] = npool.tile(
                            [128, 4, 128], F16, tag="ctxn", name="ctxnsb"
                        )
                    nc.tensor.transpose(
                        ost["t"][:, s, :], ctxnT[:, s, :, :], ident[:]
                    )
                    nc.vector.tensor_copy(
                        ost["sb"][:, s, :], ost["t"][:, s, :]
                    )
                bgq.append((("o", c), None, tback))
                for half in range(2):
                    def omm(s=s, half=half):
                        op = psO.tile(
                            [128, QC], F32, tag="mm512", name="opout"
                        )
                        if f32path:
                            lhsT, rhs = ost["sb32"][:], w_o32_sb
                        else:
                            lhsT, rhs = ost["sb"][:, s, :], w_o_sb
                        nc.tensor.matmul(
                            op[:],
                            lhsT,
                            rhs[:, half * QC:(half + 1) * QC],
                            start=True, stop=True,
                        )
                        dst = (
                            ost["ot32"][:, half * QC:(half + 1) * QC]
                            if f32path
                            else ost["ot"][:, s, half * QC:(half + 1) * QC]
                        )
                        # drain via DVE, borrowing the (idle) ACT engine
                        # for the tail chunk (gpsimd cannot touch PSUM)
                        if c == nqc - 1 and (s + half) % 2 == 1:
                            nc.scalar.activation(dst, op[:], AF.Copy)
                        else:
                            nc.vector.tensor_copy(dst, op[:])
                    bgq.append((("o", c), None, omm))
                if f32path and half == 1:
                    def flush32():
                        nc.sync.dma_start(outp32[:, :], ost["ot32"][:])
                    bgq.append((("o", c), None, flush32))
                if c == nqc - 1:
                    def flush_s(s=s):
                        eng = nc.gpsimd if s % 2 == 0 else nc.sync
                        eng.dma_start(
                            outp[c * QC + s * 128:c * QC + (s + 1) * 128, :]
                            .rearrange("(o p) d -> p o d", p=128),
                            ost["ot"][:, s:s + 1, :],
                        )
                    bgq.append((("o", c), "f" if s == 3 else None, flush_s))
                elif s == 3:
                    def flush():
                        s0 = 1 if c == 0 else 0
                        nc.gpsimd.dma_start(
                            outp[c * QC + s0 * 128:(c + 1) * QC, :].rearrange(
                                "(s p) d -> p s d", p=128
                            ),
                            ost["ot"][:, s0:, :],
                        )
                    bgq.append((("o", c), "f", flush))

            def emit_jloop(c):
                """Attention j-loop for chunk c. Normalization of each
                q-block s is emitted eagerly as its PV accumulation group
                completes (at j == 4c+s), so ctxT frees early for c+1."""
                # 8 PV accumulators of 65 cols; a matmul output cannot cross
                # a PSUM bank (512 f32), so pack 4 accumulators per bank:
                # bank s//2, offset (s%2)*130 + h*65
                ctxT = psCtx.tile([128, 2, 512], F32, tag="ctxT")

                def ctx_acc(s, h, w=65):
                    base = (s % 2) * 130 + h * 65
                    return ctxT[:, s // 2, base:base + w]

                def ctx_den(s):
                    v = ctxT[:, s // 2, (s % 2) * 130:(s % 2) * 130 + 130]
                    return v.rearrange("p (h c) -> p h c", h=2)[:, :, 64:65]

                rec = npool.tile([128, 4, 2, 1], F32, tag="rec")
                ctxnT = npool.tile([128, 4, 2, 64], F16, tag="ctxnT")
                ctxnT32 = (
                    npool.tile([128, 2, 64], F32, tag="ctxnT32", name="cT32")
                    if c == 0 else None
                )
                jmax = 4 * (c + 1)

                def qk_exp(j):
                    """Score + exp stage for tile j; emitted one iteration
                    ahead of the PV stage so PE's exp-wait never delays the
                    next QK (ACT stays saturated)."""
                    p = j - 4 * c
                    if j == 4 * c:
                        # diagonal tiles need chunk c's own k (m1) and v (m2)
                        drain_upto(("p", c), 2)
                    off = max(0, p) * KT
                    n_j, s_j = j // 4, j % 4
                    sc = psS.tile([128, 2, QC], F32, tag="sc")
                    for h in range(2):
                        nc.tensor.matmul(
                            sc[:, h, off:],
                            qk_t[n_j][:, h, 1, s_j * KT:(s_j + 1) * KT],
                            qk_t[c][:, h, 0, off:],
                            start=True, stop=True,
                        )
                    at = apool.tile([128, 2, QC], F16, name="at")
                    nc.scalar.activation(
                        at[:, :, off:], sc[:, :, off:], AF.Exp, scale=0.125
                    )
                    if p >= 0:
                        # zero the upper-triangular wedge in place:
                        # keep iff (off + q_local) - k - 128*p >= 0
                        nc.gpsimd.affine_select(
                            out=at[:, :, off:], in_=at[:, :, off:],
                            pattern=[[0, 2], [1, QC - off]],
                            compare_op=AluOpType.is_ge,
                            fill=0.0, base=off - KT * p, channel_multiplier=-1,
                        )
                    at32 = None
                    if c == 0 and j == 0:
                        # f32 path for q-block (0, 0): rows 0-127 feel fp16
                        # quantization at full |ctx| variance
                        at32 = npool.tile(
                            [128, 2, 128], F32, tag="at32", name="at32"
                        )
                        nc.scalar.activation(
                            at32[:], sc[:, :, 0:128], AF.Exp, scale=0.125
                        )
                        nc.gpsimd.affine_select(
                            out=at32[:], in_=at32[:],
                            pattern=[[0, 2], [1, 128]],
                            compare_op=AluOpType.is_ge,
                            fill=0.0, base=0, channel_multiplier=-1,
                        )
                    return at, at32

                cur = qk_exp(0)
                for j in range(jmax):
                    p = j - 4 * c
                    off = max(0, p) * KT
                    n_j, s_j = j // 4, j % 4
                    at, at32 = cur
                    if j + 1 < jmax:
                        cur = qk_exp(j + 1)
                    if at32 is not None:
                        for h in range(2):
                            nc.tensor.matmul(
                                ctx_acc(0, h),
                                at32[:, h, :],
                                v32_t[:, h * 65:(h + 1) * 65],
                                start=(h == 0), stop=True,
                                skip_group_check=True,
                            )
                    for h in range(2):
                        for s in range(max(0, p), 4):
                            if c == 0 and s == 0:
                                continue  # f32 path above
                            # start_tensor_calc zeroes the WHOLE 2KB psum
                            # bank (zero region): exactly one start per bank
                            # (first matmul of j==0); siblings inherit the
                            # pending-zero and accumulate after that.
                            nc.tensor.matmul(
                                ctx_acc(s, h),
                                at[:, h, s * KT:(s + 1) * KT],
                                v_t[n_j][:, s_j, h * 65:(h + 1) * 65],
                                start=(j == 0 and h == 0
                                       and (s == 2 or (s == 0 and c > 0))),
                                stop=(j == 4 * c + s),
                                skip_group_check=True,
                            )
                    if p >= 0:
                        # q-block s=p is complete: normalize it now and queue
                        # its share of the output projection
                        nc.vector.reciprocal(rec[:, p, :, :], ctx_den(p))
                        tgt = ctxnT32 if (c == 0 and p == 0) else None
                        for h in range(2):
                            nc.vector.tensor_scalar_mul(
                                tgt[:, h, :] if tgt is not None
                                else ctxnT[:, p, h, :],
                                ctx_acc(p, h, w=64),
                                rec[:, p, h, :],
                            )
                        push_outproj_s(c, ctxnT, p, ost, ctxnT32=tgt)
                    drain_n(2)

            xts = emit_xt_loads(0)
            push_proj(0, xts)
            for c in range(nqc):
                if c + 1 < nqc:
                    xts_next = emit_xt_loads(c + 1)
                    push_proj(c + 1, xts_next)
                # chunk c's q projection (m0, incl. head B relocation) must
                # be done before its QKs; outproj(c-2) must fully land
                # before its ctxnT/ot ring slots are reused this chunk
                if c >= 2:
                    drain_upto(("o", c - 2), "f")
                drain_upto(("p", c), 0)
                ost = {}
                emit_jloop(c)
                del ost
            drain_upto(("o", nqc - 1), "f")
    nc.compile()
    return nc


_PROGRAM_CACHE: dict = {}


def _get_program(S: int):
    if S not in _PROGRAM_CACHE:
        _PROGRAM_CACHE[S] = build_program(S)
    return _PROGRAM_CACHE[S]


def make_in_maps(x, w_qkv, b_qkv, w_out):
    x = np.asarray(x, dtype=np.float32)
    w_qkv = np.asarray(w_qkv, dtype=np.float32)
    b_qkv = np.asarray(b_qkv, dtype=np.float32)
    w_out = np.asarray(w_out, dtype=np.float32)
    S = x.shape[1]
    xT = np.ascontiguousarray(x.reshape(S, D).T)
    in_maps = []
    for c in range(N_CORES):
        lo, hi = 128 * c, 128 * (c + 1)
        w_shard = np.ascontiguousarray(
            np.concatenate(
                [w_qkv[:, lo:hi], w_qkv[:, D + lo:D + hi], w_qkv[:, 2 * D + lo:2 * D + hi]],
                axis=1,
            )
        )
        b_shard = np.concatenate(
            [b_qkv[lo:hi], b_qkv[D + lo:D + hi], b_qkv[2 * D + lo:2 * D + hi]]
        )
        w_o_shard32 = np.ascontiguousarray(w_out[lo:hi, :])
        w_o_shard = w_o_shard32.astype(F16NP)
        in_maps.append(
            {"xT": xT, "w_sh": w_shard, "b_sh": b_shard, "w_o": w_o_shard,
             "w_o32": w_o_shard32}
        )
    return in_maps


def kernel(x, w_qkv, b_qkv, w_out, b_out):
    x = np.asarray(x, dtype=np.float32)
    b_out = np.asarray(b_out, dtype=np.float32)
    B, S, _ = x.shape
    in_maps = make_in_maps(x, w_qkv, b_qkv, w_out)
    nc = _get_program(S)
    res = run_bass_kernel_spmd(nc, in_maps, list(range(N_CORES))).results
    out = res[0]["outp"].astype(np.float32)
    out32 = res[0]["outp32"].copy()
    for c in range(1, N_CORES):
        out += res[c]["outp"].astype(np.float32)
        out32 += res[c]["outp32"]
    out[0:128] = out32
    out += b_out
    return out.reshape(B, S, D)


# revision 3
# speedup vs baseline: 1.1768x; 1.1768x over previous
"""Causal self-attention (B=1, S=4096, D=1024, 16 heads) on 8 trn2 NeuronCores.

Sharding: tensor-parallel over heads (2 heads per core). Each core computes
the qkv projection for its head pair, causal attention, and a partial output
projection; the host sums the 8 fp16 partials (plus an f32 side-output for
rows 0-127) and adds b_out.

Design (vs the fp32r baseline):
- The attention j-loop is ACT-engine-gated (exp ~1.04us/tile vs PE ~0.65us),
  so PV runs TRANSPOSED in fp16: ctx^T accumulators [128q, 65] per q-block,
  moving operand = [V|1] (65 cols). This halves PV tensor rows AND puts the
  softmax denominator on the partition axis, so normalization is a plain
  per-partition tensor_scalar_mul (no partition broadcast / gpsimd chain).
- PE is in-order; projection matmuls of chunk c+1 and the output projection
  of chunk c (per q-block, as each lands) are queued as background closures
  drained between j-iterations to fill PE's exp-wait gaps. Group-selective
  barriers force-drain only what correctness needs (q before the j-loop,
  k/v before the diagonal, outproj before its buffer rings recycle).
- q/k/v/attention-weights are fp16 (fp32 PSUM accumulation everywhere;
  fp16's 10-bit mantissa matmuls run at full PE rate). Rows 0-127 attend
  few keys so |ctx| has no averaging decay there: q-block (chunk0, s=0)
  runs an f32/f32r path end-to-end and is emitted via a separate f32
  output that the host prefers.
- PSUM: start_tensor_calc zeroes a whole 2KB bank, so the 8 packed ctx^T
  accumulators get exactly one start per bank; diagonal masking runs on
  gpsimd (affine_select) off the critical engines.
"""
import sys

sys.path.insert(0, "/opt/trn_rl_repo")

from contextlib import ExitStack

import numpy as np

import concourse.tile as tile
from concourse import bacc, mybir
from concourse.alu_op_type import AluOpType
from concourse.masks import make_identity
from concourse.bass_utils import run_bass_kernel_spmd

D = 1024
N_CORES = 8
F32 = mybir.dt.float32
F32R = mybir.dt.float32r
F16 = mybir.dt.float16
AF = mybir.ActivationFunctionType
F16NP = np.float16

QC = 512  # q-chunk width
KT = 128  # k-tile width


def build_program(S: int = 4096, repeat: int = 1):
    nqc = S // QC

    nc = bacc.Bacc(None)
    xT = nc.declare_dram_parameter("xT", [D, S], F32R, isOutput=False)
    w_sh = nc.declare_dram_parameter("w_sh", [D, 384], F32R, isOutput=False)
    b_sh = nc.declare_dram_parameter("b_sh", [384], F32, isOutput=False)
    w_o = nc.declare_dram_parameter("w_o", [128, D], F16, isOutput=False)
    w_o32 = nc.declare_dram_parameter("w_o32", [128, D], F32R, isOutput=False)
    outp = nc.declare_dram_parameter("outp", [S, D], F16, isOutput=True)
    outp32 = nc.declare_dram_parameter("outp32", [128, D], F32, isOutput=True)

    with tile.TileContext(nc) as tc, ExitStack() as ctx:
        consts = ctx.enter_context(tc.tile_pool(name="consts", bufs=1))
        big = ctx.enter_context(tc.tile_pool(name="big", bufs=1))
        xpool = ctx.enter_context(tc.tile_pool(name="xp", bufs=4))
        vtpool = ctx.enter_context(tc.tile_pool(name="vt", bufs=2))
        stpool = ctx.enter_context(tc.tile_pool(name="st", bufs=3))
        apool = ctx.enter_context(tc.tile_pool(name="at", bufs=6))
        npool = ctx.enter_context(tc.tile_pool(name="nrm", bufs=4))
        opool = ctx.enter_context(tc.tile_pool(name="ot", bufs=5))
        o32pool = ctx.enter_context(tc.tile_pool(name="ot32", bufs=1))
        psS = ctx.enter_context(tc.tile_pool(name="psS", bufs=2, space="PSUM"))
        psCtx = ctx.enter_context(tc.tile_pool(name="psCtx", bufs=1, space="PSUM"))
        psO = ctx.enter_context(tc.tile_pool(name="psO", bufs=2, space="PSUM"))

        # ---- constants
        ident_f = consts.tile([128, 128], F32)
        make_identity(nc, ident_f[:])
        ident = consts.tile([128, 128], F16)
        nc.vector.tensor_copy(ident[:], ident_f[:])

        ones_bf = consts.tile([128, 8], F16)
        nc.gpsimd.memset(ones_bf[:], 1.0)

        # load order = first-use order: w_sb m0, biases, then m1/m2/w_o
        w_sb = consts.tile([128, 8, 384], F32R)
        biases = consts.tile([128, 3], F32)
        nc.sync.dma_start(
            w_sb[:, :, 0:128],
            w_sh.rearrange("(t p) m -> p t m", p=128)[:, :, 0:128],
        )
        nc.sync.dma_start(biases[:], b_sh.rearrange("(m p) -> p m", p=128))
        for m in (1, 2):
            nc.sync.dma_start(
                w_sb[:, :, m * 128:(m + 1) * 128],
                w_sh.rearrange("(t p) m -> p t m", p=128)[:, :, m * 128:(m + 1) * 128],
            )
        w_o_sb = consts.tile([128, D], F16)
        nc.sync.dma_start(w_o_sb[:], w_o[:])
        w_o32_sb = consts.tile([128, D], F32R)
        nc.sync.dma_start(w_o32_sb[:], w_o32[:])

        # per-chunk projection tiles (separate tags so attention on chunk c
        # only depends on projections of chunks <= c)
        # qk_t[n]: [64p(hd), head, q|k, seq]   v_t[n]: [128p(k), s, (head, 65)]
        qk_t = [
            big.tile([64, 2, 2, QC], F16, tag=f"qk{n}", name=f"qk{n}")
            for n in range(nqc)
        ]
        v_t = [
            big.tile([128, 4, 130], F16, tag=f"v{n}", name=f"v{n}")
            for n in range(nqc)
        ]
        # rows 0-127 attend few keys, so their |ctx| has no averaging decay
        # and fp16 quantization of the output path is the dominant error:
        # q-block (chunk 0, s=0) runs a full f32/f32r path instead
        v32_t = big.tile([128, 130], F32, name="v32t")
        nc.gpsimd.memset(
            v32_t[:].rearrange("p (g c) -> p g c", g=2)[:, :, 64:65], 1.0
        )
        for n in range(nqc):
            nc.vector.tensor_copy(
                v_t[n][:].rearrange("p t (g c) -> p t g c", g=2)[:, :, :, 64:65],
                ones_bf[:].rearrange("p (t g o) -> p t g o", g=2, o=1),
            )

        for _rep in range(repeat):
            # global background-work queue: (key, closure). The j-loop drains
            # a couple of items per iteration (filling PE's exp-wait gaps);
            # barriers force-drain through a key when correctness requires it.
            bgq = []
            done_keys = set()

            def drain_n(k):
                for _ in range(k):
                    if not bgq:
                        return
                    # projections gate the next chunk's exps; outproj is
                    # deferrable into the late chunks' exp-gated PE slack
                    idx = next(
                        (i for i, e in enumerate(bgq) if e[0][0] == "p"), 0
                    )
                    group, sub, f = bgq.pop(idx)
                    f()
                    if sub is not None:
                        done_keys.add((group, sub))

            def drain_upto(group, sub):
                """Run queued items of ONE group (in their relative order)
                through `sub`, leaving other groups queued. All items are
                data-independent across groups, and PSUM ring slots are
                allocated lazily at call time, so out-of-FIFO draining is
                safe."""
                if (group, sub) in done_keys:
                    return
                i = 0
                while i < len(bgq):
                    g, s2, f = bgq[i]
                    if g != group:
                        i += 1
                        continue
                    bgq.pop(i)
                    f()
                    if s2 is not None:
                        done_keys.add((g, s2))
                    if s2 == sub:
                        return

            def emit_xt_loads(n):
                xts = []
                src = xT.rearrange("(t p) s -> p t s", p=128)
                # chunk 0 gates the program head: split its loads finer so
                # the first projection matmul starts sooner (and keep SP
                # free for the weight loads)
                nsplit = 4 if n == 0 else 2
                for half in range(2):
                    xt = xpool.tile([128, 4, QC], F32R)
                    w = 4 // nsplit
                    for q in range(nsplit):
                        eng = nc.sync if (q % 2 == 0 and n > 0) else nc.gpsimd
                        eng.dma_start(
                            xt[:, w * q:w * (q + 1), :],
                            src[:, 4 * half + w * q:4 * half + w * (q + 1),
                                n * QC:(n + 1) * QC],
                        )
                    xts.append(xt)
                return xts

            def push_proj(n, xts):
                """Queue the projection of chunk n as individually drainable
                closures (one PE matmul each; DVE/transpose work attached to
                the last closure of each m-block). PSUM tiles are allocated
                lazily at call time so ring-slot order == emission order."""
                stage = stpool.tile([128, 2, QC], F16)
                cell = {}
                for m in range(3):
                    for t in range(8):
                        def mm(m=m, t=t):
                            if t == 0:
                                cell[m] = psO.tile(
                                    [128, QC], F32, tag="mm512", name="psproj"
                                )
                            nc.tensor.matmul(
                                cell[m][:],
                                w_sb[:, t, m * 128:(m + 1) * 128],
                                xts[t // 4][:, t % 4, :],
                                start=(t == 0),
                                stop=(t == 7),
                                skip_group_check=True,
                            )
                        bgq.append((("p", n), None, mm))

                    def tail(m=m):
                        ps = cell[m]
                        if m < 2:
                            nc.vector.tensor_scalar_add(
                                qk_t[n][:, 0, m, :], ps[0:64, :],
                                biases[0:64, m:m + 1],
                            )
                            nc.vector.tensor_scalar_add(
                                stage[64:128, m, :], ps[64:128, :],
                                biases[64:128, m:m + 1],
                            )
                            # relocate head B to partitions 0-63 right away:
                            # q (m0) unblocks the j-loop without waiting for k
                            nc.gpsimd.dma_start(
                                qk_t[n][:, 1, m:m + 1, :],
                                stage[64:128, m:m + 1, :],
                            )
                        else:
                            vt_c = vtpool.tile([128, QC], F16)
                            nc.vector.tensor_scalar_add(
                                vt_c[:], ps[:], biases[:, 2:3]
                            )
                            tr = psO.tile(
                                [128, 4, 128], F16, tag="mm512", name="tr"
                            )
                            for s in range(4):
                                nc.tensor.transpose(
                                    tr[:, s, :],
                                    vt_c[:, s * 128:(s + 1) * 128],
                                    ident[:],
                                )
                            nc.vector.tensor_copy(
                                v_t[n][:].rearrange(
                                    "p t (g c) -> p t g c", g=2
                                )[:, :, :, 0:64],
                                tr[:].rearrange("p t (g c) -> p t g c", g=2),
                            )
                            if n == 0:
                                # f32 V of k-tile 0 for the f32 (c0,s0) path
                                vt32_c = vtpool.tile(
                                    [128, 128], F32, tag="vt32", name="vt32"
                                )
                                nc.vector.tensor_scalar_add(
                                    vt32_c[:], ps[:, 0:128], biases[:, 2:3]
                                )
                                tr32 = psO.tile(
                                    [128, 128], F32, tag="mm512", name="tr32"
                                )
                                nc.tensor.transpose(
                                    tr32[:], vt32_c[:], ident_f[:]
                                )
                                nc.vector.tensor_copy(
                                    v32_t[:].rearrange(
                                        "p (g c) -> p g c", g=2
                                    )[:, :, 0:64],
                                    tr32[:].rearrange("p (g c) -> p g c", g=2),
                                )
                    bgq.append((("p", n), m, tail))

            def push_outproj_s(c, ctxnT, s, ost, ctxnT32=None):
                """Queue the transpose-back + output projection of q-block s
                of chunk c (emitted as block s's normalization lands, so the
                last chunk's output drains during its own j-loop). Block
                (c=0, s=0) uses the f32/f32r path."""
                f32path = ctxnT32 is not None

                def tback(s=s):
                    if "ot" not in ost:
                        ost["ot"] = opool.tile([128, 4, D], F16, name="ot")
                    if f32path:
                        ost["ot32"] = o32pool.tile(
                            [128, D], F32, tag="ot32", name="ot32"
                        )
                        t32 = psO.tile(
                            [128, 128], F32, tag="mm512", name="ctxnps32"
                        )
                        nc.tensor.transpose(t32[:], ctxnT32[:], ident_f[:])
                        ost["sb32"] = npool.tile(
                            [128, 128], F32R, tag="ctxn32", name="ctxnsb32"
                        )
                        nc.vector.tensor_copy(ost["sb32"][:], t32[:])
                        return
                    if "t" not in ost:
                        ost["t"] = psO.tile(
                            [128, 4, 128], F16, tag="mm512", name="ctxnps"
                        )
                        ost["sb"] = npool.tile(
                            [128, 4, 128], F16, tag="ctxn", name="ctxnsb"
                        )
                    nc.tensor.transpose(
                        ost["t"][:, s, :], ctxnT[:, s, :, :], ident[:]
                    )
                    nc.vector.tensor_copy(
                        ost["sb"][:, s, :], ost["t"][:, s, :]
                    )
                bgq.append((("o", c), None, tback))
                for half in range(2):
                    def omm(s=s, half=half):
                        op = psO.tile(
                            [128, QC], F32, tag="mm512", name="opout"
                        )
                        if f32path:
                            lhsT, rhs = ost["sb32"][:], w_o32_sb
                        else:
                            lhsT, rhs = ost["sb"][:, s, :], w_o_sb
                        nc.tensor.matmul(
                            op[:],
                            lhsT,
                            rhs[:, half * QC:(half + 1) * QC],
                            start=True, stop=True,
                        )
                        dst = (
                            ost["ot32"][:, half * QC:(half + 1) * QC]
                            if f32path
                            else ost["ot"][:, s, half * QC:(half + 1) * QC]
                        )
                        # drain via DVE, borrowing the (idle) ACT engine
                        # for the tail chunk (gpsimd cannot touch PSUM)
                        if c == nqc - 1 and (s + half) % 2 == 1:
                            nc.scalar.activation(dst, op[:], AF.Copy)
                        else:
                            nc.vector.tensor_copy(dst, op[:])
                    bgq.append((("o", c), None, omm))
                if f32path and half == 1:
                    def flush32():
                        nc.sync.dma_start(outp32[:, :], ost["ot32"][:])
                    bgq.append((("o", c), None, flush32))
                if c == nqc - 1:
                    def flush_s(s=s):
                        eng = nc.gpsimd if s % 2 == 0 else nc.sync
                        eng.dma_start(
                            outp[c * QC + s * 128:c * QC + (s + 1) * 128, :]
                            .rearrange("(o p) d -> p o d", p=128),
                            ost["ot"][:, s:s + 1, :],
                        )
                    bgq.append((("o", c), "f" if s == 3 else None, flush_s))
                elif s == 3:
                    def flush():
                        s0 = 1 if c == 0 else 0
                        nc.gpsimd.dma_start(
                            outp[c * QC + s0 * 128:(c + 1) * QC, :].rearrange(
                                "(s p) d -> p s d", p=128
                            ),
                            ost["ot"][:, s0:, :],
                        )
                    bgq.append((("o", c), "f", flush))

            def emit_jloop(c):
                """Attention j-loop for chunk c. Normalization of each
                q-block s is emitted eagerly as its PV accumulation group
                completes (at j == 4c+s), so ctxT frees early for c+1."""
                # 8 PV accumulators of 65 cols; a matmul output cannot cross
                # a PSUM bank (512 f32), so pack 4 accumulators per bank:
                # bank s//2, offset (s%2)*130 + h*65
                ctxT = psCtx.tile([128, 2, 512], F32, tag="ctxT")

                def ctx_acc(s, h, w=65):
                    base = (s % 2) * 130 + h * 65
                    return ctxT[:, s // 2, base:base + w]

                def ctx_den(s):
                    v = ctxT[:, s // 2, (s % 2) * 130:(s % 2) * 130 + 130]
                    return v.rearrange("p (h c) -> p h c", h=2)[:, :, 64:65]

                rec = npool.tile([128, 4, 2, 1], F32, tag="rec")
                ctxnT = npool.tile([128, 4, 2, 64], F16, tag="ctxnT")
                ctxnT32 = (
                    npool.tile([128, 2, 64], F32, tag="ctxnT32", name="cT32")
                    if c == 0 else None
                )
                jmax = 4 * (c + 1)

                def qk_exp(j):
                    """Score + exp stage for tile j; emitted one iteration
                    ahead of the PV stage so PE's exp-wait never delays the
                    next QK (ACT stays saturated)."""
                    p = j - 4 * c
                    if j == 4 * c:
                        # diagonal tiles need chunk c's own k (m1) and v (m2)
                        drain_upto(("p", c), 2)
                    off = max(0, p) * KT
                    n_j, s_j = j // 4, j % 4
                    sc = psS.tile([128, 2, QC], F32, tag="sc")
                    for h in range(2):
                        nc.tensor.matmul(
                            sc[:, h, off:],
                            qk_t[n_j][:, h, 1, s_j * KT:(s_j + 1) * KT],
                            qk_t[c][:, h, 0, off:],
                            start=True, stop=True,
                        )
                    at = apool.tile([128, 2, QC], F16, name="at")
                    nc.scalar.activation(
                        at[:, :, off:], sc[:, :, off:], AF.Exp, scale=0.125
                    )
                    if p >= 0:
                        # zero the upper-triangular wedge in place:
                        # keep iff (off + q_local) - k - 128*p >= 0
                        nc.gpsimd.affine_select(
                            out=at[:, :, off:], in_=at[:, :, off:],
                            pattern=[[0, 2], [1, QC - off]],
                            compare_op=AluOpType.is_ge,
                            fill=0.0, base=off - KT * p, channel_multiplier=-1,
                        )
                    at32 = None
                    if c == 0 and j == 0:
                        # f32 path for q-block (0, 0): rows 0-127 feel fp16
                        # quantization at full |ctx| variance
                        at32 = npool.tile(
                            [128, 2, 128], F32, tag="at32", name="at32"
                        )
                        nc.scalar.activation(
                            at32[:], sc[:, :, 0:128], AF.Exp, scale=0.125
                        )
                        nc.gpsimd.affine_select(
                            out=at32[:], in_=at32[:],
                            pattern=[[0, 2], [1, 128]],
                            compare_op=AluOpType.is_ge,
                            fill=0.0, base=0, channel_multiplier=-1,
                        )
                    return at, at32

                cur = qk_exp(0)
                for j in range(jmax):
                    p = j - 4 * c
                    off = max(0, p) * KT
                    n_j, s_j = j // 4, j % 4
                    at, at32 = cur
                    if j + 1 < jmax:
                        cur = qk_exp(j + 1)
                    if at32 is not None:
                        for h in range(2):
                            nc.tensor.matmul(
                                ctx_acc(0, h),
                                at32[:, h, :],
                                v32_t[:, h * 65:(h + 1) * 65],
                                start=(h == 0), stop=True,
                                skip_group_check=True,
                            )
                    for h in range(2):
                        for s in range(max(0, p), 4):
                            if c == 0 and s == 0:
                                continue  # f32 path above
                            # start_tensor_calc zeroes the WHOLE 2KB psum
                            # bank (zero region): exactly one start per bank
                            # (first matmul of j==0); siblings inherit the
                            # pending-zero and accumulate after that.
                            nc.tensor.matmul(
                                ctx_acc(s, h),
                                at[:, h, s * KT:(s + 1) * KT],
                                v_t[n_j][:, s_j, h * 65:(h + 1) * 65],
                                start=(j == 0 and h == 0
                                       and (s == 2 or (s == 0 and c > 0))),
                                stop=(j == 4 * c + s),
                                skip_group_check=True,
                            )
                    if p >= 0:
                        # q-block s=p is complete: normalize it now and queue
                        # its share of the output projection
                        nc.vector.reciprocal(rec[:, p, :, :], ctx_den(p))
                        tgt = ctxnT32 if (c == 0 and p == 0) else None
                        for h in range(2):
                            nc.vector.tensor_scalar_mul(
                                tgt[:, h, :] if tgt is not None
                                else ctxnT[:, p, h, :],
                                ctx_acc(p, h, w=64),
                                rec[:, p, h, :],
                            )
                        push_outproj_s(c, ctxnT, p, ost, ctxnT32=tgt)
                    # hold drains near the chunk boundary: deferred items
                    # would otherwise queue ahead of the next chunk's first
                    # QK on the in-order PE and starve ACT at the handoff
                    if j < jmax - 2 or c == nqc - 1:
                        drain_n(2)

            xts = emit_xt_loads(0)
            push_proj(0, xts)
            for c in range(nqc):
                if c + 1 < nqc:
                    xts_next = emit_xt_loads(c + 1)
                    push_proj(c + 1, xts_next)
                # chunk c's q projection (m0, incl. head B relocation) must
                # be done before its QKs; outproj(c-4) must fully land
                # before its ctxnT/ot ring slots are reused this chunk
                # (deep rings let early outproj work defer into the late
                # chunks' exp-gated PE slack)
                if c >= 4:
                    drain_upto(("o", c - 4), "f")
                drain_upto(("p", c), 0)
                ost = {}
                emit_jloop(c)
                del ost
            drain_upto(("o", nqc - 1), "f")
    nc.compile()
    return nc


_PROGRAM_CACHE: dict = {}


def _get_program(S: int):
    if S not in _PROGRAM_CACHE:
        _PROGRAM_CACHE[S] = build_program(S)
    return _PROGRAM_CACHE[S]


def make_in_maps(x, w_qkv, b_qkv, w_out):
    x = np.asarray(x, dtype=np.float32)
    w_qkv = np.asarray(w_qkv, dtype=np.float32)
    b_qkv = np.asarray(b_qkv, dtype=np.float32)
    w_out = np.asarray(w_out, dtype=np.float32)
    S = x.shape[1]
    xT = np.ascontiguousarray(x.reshape(S, D).T)
    in_maps = []
    for c in range(N_CORES):
        lo, hi = 128 * c, 128 * (c + 1)
        w_shard = np.ascontiguousarray(
            np.concatenate(
                [w_qkv[:, lo:hi], w_qkv[:, D + lo:D + hi], w_qkv[:, 2 * D + lo:2 * D + hi]],
                axis=1,
            )
        )
        b_shard = np.concatenate(
            [b_qkv[lo:hi], b_qkv[D + lo:D + hi], b_qkv[2 * D + lo:2 * D + hi]]
        )
        w_o_shard32 = np.ascontiguousarray(w_out[lo:hi, :])
        w_o_shard = w_o_shard32.astype(F16NP)
        in_maps.append(
            {"xT": xT, "w_sh": w_shard, "b_sh": b_shard, "w_o": w_o_shard,
             "w_o32": w_o_shard32}
        )
    return in_maps


def kernel(x, w_qkv, b_qkv, w_out, b_out):
    x = np.asarray(x, dtype=np.float32)
    b_out = np.asarray(b_out, dtype=np.float32)
    B, S, _ = x.shape
    in_maps = make_in_maps(x, w_qkv, b_qkv, w_out)
    nc = _get_program(S)
    res = run_bass_kernel_spmd(nc, in_maps, list(range(N_CORES))).results
    out = res[0]["outp"].astype(np.float32)
    out32 = res[0]["outp32"].copy()
    for c in range(1, N_CORES):
        out += res[c]["outp"].astype(np.float32)
        out32 += res[c]["outp32"]
    out[0:128] = out32
    out += b_out
    return out.reshape(B, S, D)


# revision 4
# speedup vs baseline: 1.7526x; 1.4893x over previous
"""Causal self-attention (B=1, S=4096, D=1024, 16 heads) on 8 trn2 NeuronCores.

Sharding: tensor-parallel over heads (2 heads per core). Each core computes
the qkv projection for its head pair, causal attention, and a partial output
projection; the host sums the 8 fp16 partials (plus an f32 side-output for
rows 0-127) and adds b_out.

Design (vs the fp32r baseline):
- The attention j-loop is ACT-engine-gated (exp ~1.04us/tile vs PE ~0.65us),
  so PV runs TRANSPOSED in fp16: ctx^T accumulators [128q, 65] per q-block,
  moving operand = [V|1] (65 cols). This halves PV tensor rows AND puts the
  softmax denominator on the partition axis, so normalization is a plain
  per-partition tensor_scalar_mul (no partition broadcast / gpsimd chain).
- PE is in-order; projection matmuls of chunk c+1 and the output projection
  of chunk c (per q-block, as each lands) are queued as background closures
  drained between j-iterations to fill PE's exp-wait gaps. Group-selective
  barriers force-drain only what correctness needs (q before the j-loop,
  k/v before the diagonal, outproj before its buffer rings recycle).
- q/k/v/attention-weights are fp16 (fp32 PSUM accumulation everywhere;
  fp16's 10-bit mantissa matmuls run at full PE rate). Rows 0-127 attend
  few keys so |ctx| has no averaging decay there: q-block (chunk0, s=0)
  runs an f32/f32r path end-to-end and is emitted via a separate f32
  output that the host prefers.
- PSUM: start_tensor_calc zeroes a whole 2KB bank, so the 8 packed ctx^T
  accumulators get exactly one start per bank; diagonal masking runs on
  gpsimd (affine_select) off the critical engines.
"""
import sys

sys.path.insert(0, "/opt/trn_rl_repo")

from contextlib import ExitStack

import numpy as np

import concourse.tile as tile
from concourse import bacc, mybir
from concourse.alu_op_type import AluOpType
from concourse.masks import make_identity
from concourse.bass_utils import run_bass_kernel_spmd

D = 1024
N_CORES = 8
F32 = mybir.dt.float32
F32R = mybir.dt.float32r
F16 = mybir.dt.float16
AF = mybir.ActivationFunctionType
F16NP = np.float16

QC = 512  # q-chunk width
KT = 128  # k-tile width


def build_program(S: int = 4096, repeat: int = 1):
    nqc = S // QC

    nc = bacc.Bacc(None)
    xT = nc.declare_dram_parameter("xT", [D, S], F32R, isOutput=False)
    w_sh = nc.declare_dram_parameter("w_sh", [D, 384], F32R, isOutput=False)
    b_sh = nc.declare_dram_parameter("b_sh", [384], F32, isOutput=False)
    w_o = nc.declare_dram_parameter("w_o", [128, D], F16, isOutput=False)
    w_o32 = nc.declare_dram_parameter("w_o32", [128, D], F32R, isOutput=False)
    outp = nc.declare_dram_parameter("outp", [S, D], F16, isOutput=True)
    outp32 = nc.declare_dram_parameter("outp32", [128, D], F32, isOutput=True)

    with tile.TileContext(nc) as tc, ExitStack() as ctx:
        consts = ctx.enter_context(tc.tile_pool(name="consts", bufs=1))
        big = ctx.enter_context(tc.tile_pool(name="big", bufs=1))
        xpool = ctx.enter_context(tc.tile_pool(name="xp", bufs=4))
        vtpool = ctx.enter_context(tc.tile_pool(name="vt", bufs=2))
        stpool = ctx.enter_context(tc.tile_pool(name="st", bufs=3))
        apool = ctx.enter_context(tc.tile_pool(name="at", bufs=6))
        npool = ctx.enter_context(tc.tile_pool(name="nrm", bufs=4))
        opool = ctx.enter_context(tc.tile_pool(name="ot", bufs=5))
        o32pool = ctx.enter_context(tc.tile_pool(name="ot32", bufs=1))
        psS = ctx.enter_context(tc.tile_pool(name="psS", bufs=2, space="PSUM"))
        psCtx = ctx.enter_context(tc.tile_pool(name="psCtx", bufs=1, space="PSUM"))
        psO = ctx.enter_context(tc.tile_pool(name="psO", bufs=2, space="PSUM"))

        # ---- constants
        ident_f = consts.tile([128, 128], F32)
        make_identity(nc, ident_f[:])
        ident = consts.tile([128, 128], F16)
        nc.vector.tensor_copy(ident[:], ident_f[:])

        ones_bf = consts.tile([128, 8], F16)
        nc.gpsimd.memset(ones_bf[:], 1.0)

        # load order = first-use order: w_sb m0, biases, then m1/m2/w_o
        w_sb = consts.tile([128, 8, 384], F32R)
        biases = consts.tile([128, 3], F32)
        nc.sync.dma_start(
            w_sb[:, :, 0:128],
            w_sh.rearrange("(t p) m -> p t m", p=128)[:, :, 0:128],
        )
        nc.sync.dma_start(biases[:], b_sh.rearrange("(m p) -> p m", p=128))
        for m in (1, 2):
            nc.sync.dma_start(
                w_sb[:, :, m * 128:(m + 1) * 128],
                w_sh.rearrange("(t p) m -> p t m", p=128)[:, :, m * 128:(m + 1) * 128],
            )
        w_o_sb = consts.tile([128, D], F16)
        nc.sync.dma_start(w_o_sb[:], w_o[:])
        w_o32_sb = consts.tile([128, D], F32R)
        nc.sync.dma_start(w_o32_sb[:], w_o32[:])

        # per-chunk projection tiles (separate tags so attention on chunk c
        # only depends on projections of chunks <= c)
        # qk_t[n]: [64p(hd), head, q|k, seq]   v_t[n]: [128p(k), s, (head, 65)]
        qk_t = [
            big.tile([64, 2, 2, QC], F16, tag=f"qk{n}", name=f"qk{n}")
            for n in range(nqc)
        ]
        v_t = [
            big.tile([128, 4, 130], F16, tag=f"v{n}", name=f"v{n}")
            for n in range(nqc)
        ]
        # rows 0-127 attend few keys, so their |ctx| has no averaging decay
        # and fp16 quantization of the output path is the dominant error:
        # q-block (chunk 0, s=0) runs a full f32/f32r path instead
        v32_t = big.tile([128, 130], F32, name="v32t")
        nc.gpsimd.memset(
            v32_t[:].rearrange("p (g c) -> p g c", g=2)[:, :, 64:65], 1.0
        )
        for n in range(nqc):
            nc.vector.tensor_copy(
                v_t[n][:].rearrange("p t (g c) -> p t g c", g=2)[:, :, :, 64:65],
                ones_bf[:].rearrange("p (t g o) -> p t g o", g=2, o=1),
            )

        for _rep in range(repeat):
            # global background-work queue: (key, closure). The j-loop drains
            # a couple of items per iteration (filling PE's exp-wait gaps);
            # barriers force-drain through a key when correctness requires it.
            bgq = []
            done_keys = set()

            def drain_n(k):
                for _ in range(k):
                    if not bgq:
                        return
                    # projections gate the next chunk's exps; outproj is
                    # deferrable into the late chunks' exp-gated PE slack
                    idx = next(
                        (i for i, e in enumerate(bgq) if e[0][0] == "p"), 0
                    )
                    group, sub, f = bgq.pop(idx)
                    f()
                    if sub is not None:
                        done_keys.add((group, sub))

            def drain_upto(group, sub):
                """Run queued items of ONE group (in their relative order)
                through `sub`, leaving other groups queued. All items are
                data-independent across groups, and PSUM ring slots are
                allocated lazily at call time, so out-of-FIFO draining is
                safe."""
                if (group, sub) in done_keys:
                    return
                i = 0
                while i < len(bgq):
                    g, s2, f = bgq[i]
                    if g != group:
                        i += 1
                        continue
                    bgq.pop(i)
                    f()
                    if s2 is not None:
                        done_keys.add((g, s2))
                    if s2 == sub:
                        return

            def emit_xt_loads(n):
                xts = []
                src = xT.rearrange("(t p) s -> p t s", p=128)
                # chunk 0 gates the program head: split its loads finer so
                # the first projection matmul starts sooner (and keep SP
                # free for the weight loads)
                nsplit = 4 if n == 0 else 2
                for half in range(2):
                    xt = xpool.tile([128, 4, QC], F32R)
                    w = 4 // nsplit
                    for q in range(nsplit):
                        eng = nc.sync if (q % 2 == 0 and n > 0) else nc.gpsimd
                        eng.dma_start(
                            xt[:, w * q:w * (q + 1), :],
                            src[:, 4 * half + w * q:4 * half + w * (q + 1),
                                n * QC:(n + 1) * QC],
                        )
                    xts.append(xt)
                return xts

            def push_proj(n, xts):
                """Queue the projection of chunk n as individually drainable
                closures (one PE matmul each; DVE/transpose work attached to
                the last closure of each m-block). PSUM tiles are allocated
                lazily at call time so ring-slot order == emission order."""
                stage = stpool.tile([128, 2, QC], F16)
                cell = {}
                for m in range(3):
                    for t in range(8):
                        def mm(m=m, t=t):
                            if t == 0:
                                cell[m] = psO.tile(
                                    [128, QC], F32, tag="mm512", name="psproj"
                                )
                            nc.tensor.matmul(
                                cell[m][:],
                                w_sb[:, t, m * 128:(m + 1) * 128],
                                xts[t // 4][:, t % 4, :],
                                start=(t == 0),
                                stop=(t == 7),
                                skip_group_check=True,
                            )
                        bgq.append((("p", n), None, mm))

                    def tail(m=m):
                        ps = cell[m]
                        if m < 2:
                            nc.vector.tensor_scalar_add(
                                qk_t[n][:, 0, m, :], ps[0:64, :],
                                biases[0:64, m:m + 1],
                            )
                            nc.vector.tensor_scalar_add(
                                stage[64:128, m, :], ps[64:128, :],
                                biases[64:128, m:m + 1],
                            )
                            # relocate head B to partitions 0-63 right away:
                            # q (m0) unblocks the j-loop without waiting for k
                            nc.gpsimd.dma_start(
                                qk_t[n][:, 1, m:m + 1, :],
                                stage[64:128, m:m + 1, :],
                            )
                        else:
                            vt_c = vtpool.tile([128, QC], F16)
                            nc.vector.tensor_scalar_add(
                                vt_c[:], ps[:], biases[:, 2:3]
                            )
                            tr = psO.tile(
                                [128, 4, 128], F16, tag="mm512", name="tr"
                            )
                            for s in range(4):
                                nc.tensor.transpose(
                                    tr[:, s, :],
                                    vt_c[:, s * 128:(s + 1) * 128],
                                    ident[:],
                                )
                            nc.vector.tensor_copy(
                                v_t[n][:].rearrange(
                                    "p t (g c) -> p t g c", g=2
                                )[:, :, :, 0:64],
                                tr[:].rearrange("p t (g c) -> p t g c", g=2),
                            )
                            if n == 0:
                                # f32 V of k-tile 0 for the f32 (c0,s0) path
                                vt32_c = vtpool.tile(
                                    [128, 128], F32, tag="vt32", name="vt32"
                                )
                                nc.vector.tensor_scalar_add(
                                    vt32_c[:], ps[:, 0:128], biases[:, 2:3]
                                )
                                tr32 = psO.tile(
                                    [128, 128], F32, tag="mm512", name="tr32"
                                )
                                nc.tensor.transpose(
                                    tr32[:], vt32_c[:], ident_f[:]
                                )
                                nc.vector.tensor_copy(
                                    v32_t[:].rearrange(
                                        "p (g c) -> p g c", g=2
                                    )[:, :, 0:64],
                                    tr32[:].rearrange("p (g c) -> p g c", g=2),
                                )
                    bgq.append((("p", n), m, tail))

            def push_outproj_s(c, ctxnT, s, ost, ctxnT32=None):
                """Queue the transpose-back + output projection of q-block s
                of chunk c (emitted as block s's normalization lands, so the
                last chunk's output drains during its own j-loop). Block
                (c=0, s=0) uses the f32/f32r path."""
                f32path = ctxnT32 is not None

                def tback(s=s):
                    if "ot" not in ost:
                        ost["ot"] = opool.tile([128, 4, D], F16, name="ot")
                    if f32path:
                        ost["ot32"] = o32pool.tile(
                            [128, D], F32, tag="ot32", name="ot32"
                        )
                        t32 = psO.tile(
                            [128, 128], F32, tag="mm512", name="ctxnps32"
                        )
                        nc.tensor.transpose(t32[:], ctxnT32[:], ident_f[:])
                        ost["sb32"] = npool.tile(
                            [128, 128], F32R, tag="ctxn32", name="ctxnsb32"
                        )
                        nc.vector.tensor_copy(ost["sb32"][:], t32[:])
                        return
                    if "t" not in ost:
                        ost["t"] = psO.tile(
                            [128, 4, 128], F16, tag="mm512", name="ctxnps"
                        )
                        ost["sb"] = npool.tile(
                            [128, 4, 128], F16, tag="ctxn", name="ctxnsb"
                        )
                    nc.tensor.transpose(
                        ost["t"][:, s, :], ctxnT[:, s, :, :], ident[:]
                    )
                    nc.vector.tensor_copy(
                        ost["sb"][:, s, :], ost["t"][:, s, :]
                    )
                bgq.append((("o", c), None, tback))
                for half in range(2):
                    def omm(s=s, half=half):
                        op = psO.tile(
                            [128, QC], F32, tag="mm512", name="opout"
                        )
                        if f32path:
                            lhsT, rhs = ost["sb32"][:], w_o32_sb
                        else:
                            lhsT, rhs = ost["sb"][:, s, :], w_o_sb
                        nc.tensor.matmul(
                            op[:],
                            lhsT,
                            rhs[:, half * QC:(half + 1) * QC],
                            start=True, stop=True,
                        )
                        dst = (
                            ost["ot32"][:, half * QC:(half + 1) * QC]
                            if f32path
                            else ost["ot"][:, s, half * QC:(half + 1) * QC]
                        )
                        # drain via DVE, borrowing the (idle) ACT engine
                        # for the tail chunk (gpsimd cannot touch PSUM)
                        if c == nqc - 1 and (s + half) % 2 == 1:
                            nc.scalar.activation(dst, op[:], AF.Copy)
                        else:
                            nc.vector.tensor_copy(dst, op[:])
                    bgq.append((("o", c), None, omm))
                if f32path and half == 1:
                    def flush32():
                        nc.sync.dma_start(outp32[:, :], ost["ot32"][:])
                    bgq.append((("o", c), None, flush32))
                if c == nqc - 1:
                    def flush_s(s=s):
                        eng = nc.gpsimd if s % 2 == 0 else nc.sync
                        eng.dma_start(
                            outp[c * QC + s * 128:c * QC + (s + 1) * 128, :]
                            .rearrange("(o p) d -> p o d", p=128),
                            ost["ot"][:, s:s + 1, :],
                        )
                    bgq.append((("o", c), "f" if s == 3 else None, flush_s))
                elif s == 3:
                    def flush():
                        s0 = 1 if c == 0 else 0
                        nc.gpsimd.dma_start(
                            outp[c * QC + s0 * 128:(c + 1) * QC, :].rearrange(
                                "(s p) d -> p s d", p=128
                            ),
                            ost["ot"][:, s0:, :],
                        )
                    bgq.append((("o", c), "f", flush))

            def emit_jloop(c):
                """Attention j-loop for chunk c. Normalization of each
                q-block s is emitted eagerly as its PV accumulation group
                completes (at j == 4c+s), so ctxT frees early for c+1."""
                # 8 PV accumulators of 65 cols; a matmul output cannot cross
                # a PSUM bank (512 f32), so pack 4 accumulators per bank:
                # bank s//2, offset (s%2)*130 + h*65
                ctxT = psCtx.tile([128, 2, 512], F32, tag="ctxT")

                def ctx_acc(s, h, w=65):
                    base = (s % 2) * 130 + h * 65
                    return ctxT[:, s // 2, base:base + w]

                def ctx_den(s):
                    v = ctxT[:, s // 2, (s % 2) * 130:(s % 2) * 130 + 130]
                    return v.rearrange("p (h c) -> p h c", h=2)[:, :, 64:65]

                rec = npool.tile([128, 4, 2, 1], F32, tag="rec")
                ctxnT = npool.tile([128, 4, 2, 64], F16, tag="ctxnT")
                ctxnT32 = (
                    npool.tile([128, 2, 64], F32, tag="ctxnT32", name="cT32")
                    if c == 0 else None
                )
                jmax = 4 * (c + 1)

                def qk_exp(j):
                    """Score + exp stage for tile j; emitted one iteration
                    ahead of the PV stage so PE's exp-wait never delays the
                    next QK (ACT stays saturated)."""
                    p = j - 4 * c
                    if j == 4 * c:
                        # diagonal tiles need chunk c's own k (m1) and v (m2)
                        drain_upto(("p", c), 2)
                    off = max(0, p) * KT
                    n_j, s_j = j // 4, j % 4
                    sc = psS.tile([128, 2, QC], F32, tag="sc")
                    for h in range(2):
                        nc.tensor.matmul(
                            sc[:, h, off:],
                            qk_t[n_j][:, h, 1, s_j * KT:(s_j + 1) * KT],
                            qk_t[c][:, h, 0, off:],
                            start=True, stop=True,
                        )
                    at = apool.tile([128, 2, QC], F16, name="at")
                    nc.scalar.activation(
                        at[:, :, off:], sc[:, :, off:], AF.Exp, scale=0.125
                    )
                    if p >= 0:
                        # zero the upper-triangular wedge in place:
                        # keep iff (off + q_local) - k - 128*p >= 0
                        nc.gpsimd.affine_select(
                            out=at[:, :, off:], in_=at[:, :, off:],
                            pattern=[[0, 2], [1, QC - off]],
                            compare_op=AluOpType.is_ge,
                            fill=0.0, base=off - KT * p, channel_multiplier=-1,
                        )
                    at32 = None
                    if c == 0 and j == 0:
                        # f32 path for q-block (0, 0): rows 0-127 feel fp16
                        # quantization at full |ctx| variance
                        at32 = npool.tile(
                            [128, 2, 128], F32, tag="at32", name="at32"
                        )
                        nc.scalar.activation(
                            at32[:], sc[:, :, 0:128], AF.Exp, scale=0.125
                        )
                        nc.gpsimd.affine_select(
                            out=at32[:], in_=at32[:],
                            pattern=[[0, 2], [1, 128]],
                            compare_op=AluOpType.is_ge,
                            fill=0.0, base=0, channel_multiplier=-1,
                        )
                    return at, at32

                cur = qk_exp(0)
                for j in range(jmax):
                    p = j - 4 * c
                    off = max(0, p) * KT
                    n_j, s_j = j // 4, j % 4
                    at, at32 = cur
                    if j + 1 < jmax:
                        cur = qk_exp(j + 1)
                    if at32 is not None:
                        for h in range(2):
                            nc.tensor.matmul(
                                ctx_acc(0, h),
                                at32[:, h, :],
                                v32_t[:, h * 65:(h + 1) * 65],
                                start=(h == 0), stop=True,
                                skip_group_check=True,
                            )
                    for h in range(2):
                        for s in range(max(0, p), 4):
                            if c == 0 and s == 0:
                                continue  # f32 path above
                            # start_tensor_calc zeroes the WHOLE 2KB psum
                            # bank (zero region): exactly one start per bank
                            # (first matmul of j==0); siblings inherit the
                            # pending-zero and accumulate after that.
                            nc.tensor.matmul(
                                ctx_acc(s, h),
                                at[:, h, s * KT:(s + 1) * KT],
                                v_t[n_j][:, s_j, h * 65:(h + 1) * 65],
                                start=(j == 0 and h == 0
                                       and (s == 2 or (s == 0 and c > 0))),
                                stop=(j == 4 * c + s),
                                skip_group_check=True,
                            )
                    if p >= 0:
                        # q-block s=p is complete: normalize it now and queue
                        # its share of the output projection
                        nc.vector.reciprocal(rec[:, p, :, :], ctx_den(p))
                        tgt = ctxnT32 if (c == 0 and p == 0) else None
                        for h in range(2):
                            nc.vector.tensor_scalar_mul(
                                tgt[:, h, :] if tgt is not None
                                else ctxnT[:, p, h, :],
                                ctx_acc(p, h, w=64),
                                rec[:, p, h, :],
                            )
                        push_outproj_s(c, ctxnT, p, ost, ctxnT32=tgt)
                    # hold drains near the chunk boundary: deferred items
                    # would otherwise queue ahead of the next chunk's first
                    # QK on the in-order PE and starve ACT at the handoff
                    if j < jmax - 2 or c == nqc - 1:
                        drain_n(1 if (c >= 4 and c < nqc - 1) else 2)

            xts = emit_xt_loads(0)
            push_proj(0, xts)
            for c in range(nqc):
                if c + 1 < nqc:
                    xts_next = emit_xt_loads(c + 1)
                    push_proj(c + 1, xts_next)
                # chunk c's q projection (m0, incl. head B relocation) must
                # be done before its QKs; outproj(c-4) must fully land
                # before its ctxnT/ot ring slots are reused this chunk
                # (deep rings let early outproj work defer into the late
                # chunks' exp-gated PE slack)
                if c >= 4:
                    drain_upto(("o", c - 4), "f")
                drain_upto(("p", c), 0)
                ost = {}
                emit_jloop(c)
                del ost
            drain_upto(("o", nqc - 1), "f")
    nc.compile()
    return nc


_PROGRAM_CACHE: dict = {}


def _get_program(S: int):
    if S not in _PROGRAM_CACHE:
        _PROGRAM_CACHE[S] = build_program(S)
    return _PROGRAM_CACHE[S]


def make_in_maps(x, w_qkv, b_qkv, w_out):
    x = np.asarray(x, dtype=np.float32)
    w_qkv = np.asarray(w_qkv, dtype=np.float32)
    b_qkv = np.asarray(b_qkv, dtype=np.float32)
    w_out = np.asarray(w_out, dtype=np.float32)
    S = x.shape[1]
    xT = np.ascontiguousarray(x.reshape(S, D).T)
    in_maps = []
    for c in range(N_CORES):
        lo, hi = 128 * c, 128 * (c + 1)
        w_shard = np.ascontiguousarray(
            np.concatenate(
                [w_qkv[:, lo:hi], w_qkv[:, D + lo:D + hi], w_qkv[:, 2 * D + lo:2 * D + hi]],
                axis=1,
            )
        )
        b_shard = np.concatenate(
            [b_qkv[lo:hi], b_qkv[D + lo:D + hi], b_qkv[2 * D + lo:2 * D + hi]]
        )
        w_o_shard32 = np.ascontiguousarray(w_out[lo:hi, :])
        w_o_shard = w_o_shard32.astype(F16NP)
        in_maps.append(
            {"xT": xT, "w_sh": w_shard, "b_sh": b_shard, "w_o": w_o_shard,
             "w_o32": w_o_shard32}
        )
    return in_maps


def kernel(x, w_qkv, b_qkv, w_out, b_out):
    x = np.asarray(x, dtype=np.float32)
    b_out = np.asarray(b_out, dtype=np.float32)
    B, S, _ = x.shape
    in_maps = make_in_maps(x, w_qkv, b_qkv, w_out)
    nc = _get_program(S)
    res = run_bass_kernel_spmd(nc, in_maps, list(range(N_CORES))).results
    out = res[0]["outp"].astype(np.float32)
    out32 = res[0]["outp32"].copy()
    for c in range(1, N_CORES):
        out += res[c]["outp"].astype(np.float32)
        out32 += res[c]["outp32"]
    out[0:128] = out32
    out += b_out
    return out.reshape(B, S, D)
